# revision 1
# baseline (speedup 1.0000x reference)
"""DD-RoPE kernel for 8x TRN2 NeuronCores — "t-on-partitions" design.

Reference computation (B=4, T=4096, D=2048, P=256):
    deltas = einsum('btd,pd->btp', x, W) + b     # (B, T, P)
    angles = cumsum(deltas, axis=1)
    out = concat([x1*cos(a) - x2*sin(a), x2*cos(a) + x1*sin(a), x[..., 512:]], -1)

Sharding: 8 shards = 4 batches x 2 T-halves (2048 steps each), data-parallel.
The cumsum is split into independent 128-step blocks via host-computed fp64
block bases (exact cumulative angle at each 128-step boundary), so per-delta
rounding error amplifies by at most sqrt(128) and no cross-core communication
is needed.

Everything heavy runs on the PE in [time-partition, pair-free] layout:
    deltas block [128t, 256p] = sum_dc xT_chunk[128d,128t]^T @ W_chunk[128d,256p]
        (x^T chunks are the STATIONARY operand, fp16 single pass)
    angles = U^T @ fp16(deltas) + ones*base_hi/lo + ramp*b_hi/lo
        (U = upper-triangular ones: the per-block cumsum is ONE matmul;
         the rank-4 affine matmul injects the host base and the per-step
         bias t*b exactly — no DVE scan instructions at all)
    trig: magic-number range reduction to rs in [-0.5, 0.5] turns;
        sin = Sin(2pi*rs) on ScalarE; cos = Sin(pi/2 - 2pi*|rs|) reusing the
        SAME reduction (|rs| via one DVE abs_max) — 4 ScalarE passes total
    rotation on DVE in all-fp16 (2x perf mode), wide [128, 1024] tiles
        spanning 4 blocks to amortize instruction/init overheads
    out written fp16 (well within tolerance), host upcasts + passthrough.
"""

import sys

if "/opt/trn_rl_repo" not in sys.path:
    sys.path.insert(0, "/opt/trn_rl_repo")

from contextlib import ExitStack

import numpy as np

import concourse.bacc as bacc
import concourse.bass as bass
import concourse.mybir as mybir
import concourse.tile as tile
from concourse.bass_utils import run_bass_kernel_spmd

F32 = mybir.dt.float32
F16 = mybir.dt.float16
ADD = mybir.AluOpType.add
SUB = mybir.AluOpType.subtract
MULT = mybir.AluOpType.mult
MAX = mybir.AluOpType.max
IDENT = mybir.ActivationFunctionType.Identity
SIN = mybir.ActivationFunctionType.Sin
ABS = mybir.ActivationFunctionType.Abs

D = 2048          # input feature dim (contraction)
P = 256           # delta-pairs dim
ROT = 2 * P       # rotated columns (512)
TL = 2048         # time steps per shard
BK = 128          # cumsum block (base injection granularity)
NBK = TL // BK    # blocks per shard (16)
G = 4             # blocks per group (wide-tile span)
NG = NBK // G     # groups per shard (4)
KC = D // 128     # contraction chunks (16)
WID = G * P       # wide-tile columns (1024)
GS = [4, 4, 4, 2, 1, 1]   # device group sizes: small tail groups keep the
                          # final serial trig/rotate chain short
N_CORES = 8

MAGIC = 12582912.0          # 1.5 * 2**23: fp32 round-to-int magic constant
SCALE_2PI = 6.28310         # slightly < 2*pi so Sin args stay inside [-pi, pi]
HALF_PI = 1.5707964


def build_program() -> bass.Bass:
    nc = bacc.Bacc("TRN2", target_bir_lowering=False, debug=False)

    # x^T tiles, row block g: [128 d-part, bkl*KC*128 + dc*128 + t_local]
    xt = nc.dram_tensor("xt", [NG * 128, G * KC * 128], F16,
                        kind="ExternalInput").ap()
    # W, d-chunks along free: [128 d-part, dc*P + p] fp16
    w = nc.dram_tensor("w", [128, KC * P], F16, kind="ExternalInput").ap()
    # upper-triangular ones (u[t, t'] = 1 iff t <= t')
    u = nc.dram_tensor("u", [128, 128], F16, kind="ExternalInput").ap()
    # affine stationary: rows [ones, ones, ramp(1..128), ramp]
    afs = nc.dram_tensor("afs", [4, 128], F16, kind="ExternalInput").ap()
    # affine moving: rows [base_hi[bk,p], base_lo, b_hi, b_lo], bk-major
    afm = nc.dram_tensor("afm", [4, NBK * P], F16, kind="ExternalInput").ap()
    # natural-layout rotation operands, row block g:
    # [t_local, half*WID + bkl*P + p] fp16
    x12 = nc.dram_tensor("x12", [NG * 128, 2 * WID], F16,
                         kind="ExternalInput").ap()
    # rotated output, same indexing as x12
    outT = nc.dram_tensor("outT", [NG * 128, 2 * WID], F16,
                          kind="ExternalOutput").ap()

    with tile.TileContext(nc) as tc, ExitStack() as ctx:
        const_pool = ctx.enter_context(tc.tile_pool(name="const", bufs=1))
        w_pool = ctx.enter_context(tc.tile_pool(name="w", bufs=1))
        xt_pool = ctx.enter_context(tc.tile_pool(name="xt", bufs=3))
        x12_pool = ctx.enter_context(tc.tile_pool(name="x12", bufs=3))
        dp_pool = ctx.enter_context(
            tc.tile_pool(name="dp_psum", bufs=2, space="PSUM"))
        ang_pool = ctx.enter_context(
            tc.tile_pool(name="ang_psum", bufs=2, space="PSUM"))
        d16_pool = ctx.enter_context(tc.tile_pool(name="d16", bufs=2))
        a32_pool = ctx.enter_context(tc.tile_pool(name="a32", bufs=2))
        trig_pool = ctx.enter_context(tc.tile_pool(name="trig", bufs=2))
        rot_pool = ctx.enter_context(tc.tile_pool(name="rot", bufs=2))
        out_pool = ctx.enter_context(tc.tile_pool(name="out", bufs=2))

        # w first (in halves, so the first delta matmuls only wait for
        # half the weights); everything else defers behind the first x tiles
        w_sb = w_pool.tile([128, KC * P], F16, tag="w")
        nc.sync.dma_start(w_sb[:, 0:KC * P // 2], w[:, 0:KC * P // 2])
        nc.sync.dma_start(w_sb[:, KC * P // 2:], w[:, KC * P // 2:])
        u_sb = const_pool.tile([128, 128], F16, tag="u")
        afs_sb = const_pool.tile([4, 128], F16, tag="afs")
        afm_sb = const_pool.tile([4, NBK * P], F16, tag="afm")
        magic_sb = const_pool.tile([128, 1], F32, tag="magic")
        nc.gpsimd.memset(magic_sb[:], MAGIC)
        hpi_sb = const_pool.tile([128, 1], F32, tag="hpi")
        nc.gpsimd.memset(hpi_sb[:], HALF_PI)

        def angle_and_rotate(bo, gs, d16, x12t):
            """Angle matmuls + trig + rotation + out DMA for a group of gs
            blocks at block offset bo.

            Issued one group late so the PE's in-order queue never stalls
            on the Act delta-copy: while Act produces d16(g), the PE is
            already streaming the delta matmuls of group g+1.
            """
            wid = gs * P
            ang = ang_pool.tile([128, wid], F32, tag="ang")
            for bkl in range(gs):
                bk = bo + bkl
                sl = slice(bkl * P, (bkl + 1) * P)
                nc.tensor.matmul(ang[:, sl], u_sb[:], d16[:, sl],
                                 start=True, stop=False)
                nc.tensor.matmul(ang[:, sl], afs_sb[:],
                                 afm_sb[:, bk * P:(bk + 1) * P],
                                 start=False, stop=True)

            # range reduction (turns): rs = y - round(y) in [-0.5, 0.5]
            a_s = a32_pool.tile([128, wid], F32, tag="a_s")
            nc.scalar.activation(a_s[:], ang[:], IDENT,
                                 bias=magic_sb[:], scale=-1.0)
            rs = trig_pool.tile([128, wid], F16, tag="rs")
            nc.vector.scalar_tensor_tensor(rs[:], a_s[:], MAGIC, ang[:],
                                           op0=SUB, op1=ADD)
            sn = trig_pool.tile([128, wid], F16, tag="sn")
            nc.scalar.activation(sn[:], rs[:], SIN, scale=SCALE_2PI)
            # cos(2pi*y) = sin(pi/2 - 2pi*|rs|), same reduction
            # (|rs| = max(-rs, rs) on DVE, keeping ScalarE at 4 passes)
            ra = trig_pool.tile([128, wid], F16, tag="ra")
            nc.vector.scalar_tensor_tensor(ra[:], rs[:], -1.0, rs[:],
                                           op0=MULT, op1=MAX)
            cs = trig_pool.tile([128, wid], F16, tag="cs")
            nc.scalar.activation(cs[:], ra[:], SIN,
                                 scale=-SCALE_2PI, bias=hpi_sb[:])

            # rotation, all-fp16 on DVE
            x1 = x12t[:, 0:wid]
            x2 = x12t[:, wid:2 * wid]
            o = out_pool.tile([128, 2 * wid], F16, tag="o")
            t1 = rot_pool.tile([128, wid], F16, tag="t1")
            nc.vector.tensor_mul(t1[:], x1, cs[:])
            t2 = rot_pool.tile([128, wid], F16, tag="t2")
            nc.vector.tensor_mul(t2[:], x2, sn[:])
            nc.vector.tensor_sub(o[:, 0:wid], t1[:], t2[:])
            t3 = rot_pool.tile([128, wid], F16, tag="t3")
            nc.vector.tensor_mul(t3[:], x2, cs[:])
            t4 = rot_pool.tile([128, wid], F16, tag="t4")
            nc.vector.tensor_mul(t4[:], x1, sn[:])
            nc.vector.tensor_add(o[:, wid:2 * wid], t3[:], t4[:])

            # out DMA: full row for 4-block groups, 2 column slices else
            row, lo = bo // G, (bo % G) * P
            if gs == G:
                nc.sync.dma_start(outT[row * 128:(row + 1) * 128, :], o[:])
            else:
                nc.sync.dma_start(
                    outT[row * 128:(row + 1) * 128, lo:lo + wid],
                    o[:, 0:wid])
                nc.sync.dma_start(
                    outT[row * 128:(row + 1) * 128, WID + lo:WID + lo + wid],
                    o[:, wid:2 * wid])

        pend = None  # (bo, gs, d16, x12t) awaiting its angle stage
        bo = 0
        for gi, gs in enumerate(GS):
            wid = gs * P
            row, lo = bo // G, bo % G
            # x^T tile: gs blocks; group 0's DMA is split per block so the
            # first delta matmuls can start as early as possible
            xtg = xt_pool.tile([128, gs * KC * 128], F16, tag="xt")
            xsl = slice(lo * KC * 128, (lo + gs) * KC * 128)
            if gi == 0:
                for bkl in range(gs):
                    csl = slice(bkl * KC * 128, (bkl + 1) * KC * 128)
                    nc.sync.dma_start(xtg[:, csl], xt[0:128, csl])
            else:
                nc.sync.dma_start(xtg[:],
                                  xt[row * 128:(row + 1) * 128, xsl])
            # rotation operands (needed one stage later than xtg)
            x12t = x12_pool.tile([128, 2 * wid], F16, tag="x12")
            if gs == G:
                nc.sync.dma_start(x12t[:],
                                  x12[row * 128:(row + 1) * 128, :])
            else:
                nc.sync.dma_start(
                    x12t[:, 0:wid],
                    x12[row * 128:(row + 1) * 128, lo * P:lo * P + wid])
                nc.sync.dma_start(
                    x12t[:, wid:2 * wid],
                    x12[row * 128:(row + 1) * 128,
                        WID + lo * P:WID + lo * P + wid])
            if gi == 0:
                nc.sync.dma_start(u_sb[:], u[:])
                nc.sync.dma_start(afs_sb[:], afs[:])
                nc.sync.dma_start(afm_sb[:], afm[:])

            # deltas^T: [128 t, bkl*P + p] in PSUM
            dp = dp_pool.tile([128, wid], F32, tag="dp")
            for bkl in range(gs):
                sl = slice(bkl * P, (bkl + 1) * P)
                for dc in range(KC):
                    nc.tensor.matmul(
                        dp[:, sl],
                        xtg[:, (bkl * KC + dc) * 128:(bkl * KC + dc + 1) * 128],
                        w_sb[:, dc * P:(dc + 1) * P],
                        start=(dc == 0), stop=(dc == KC - 1))

            # fp16 copy of deltas (moving operand of the cumsum matmul)
            d16 = d16_pool.tile([128, wid], F16, tag="d16")
            nc.scalar.activation(d16[:], dp[:], IDENT)

            if pend is not None:
                angle_and_rotate(*pend)
            pend = (bo, gs, d16, x12t)
            bo += gs
        angle_and_rotate(*pend)

    nc.compile()
    return nc


_NC_CACHE: dict = {}


def _get_nc():
    if "nc" not in _NC_CACHE:
        _NC_CACHE["nc"] = build_program()
    return _NC_CACHE["nc"]


def prepare_weights(W: np.ndarray, b: np.ndarray):
    inv2pi = 1.0 / (2.0 * np.pi)
    Wt = W.astype(np.float64).T * inv2pi                       # [D, P]
    wh = Wt.astype(np.float16)
    bt = b.astype(np.float64) * inv2pi                         # [P]
    bh = bt.astype(np.float16)
    bl = (bt - bh.astype(np.float64)).astype(np.float16)
    # [D, P] -> [128, KC*P] with d-chunks along the free dim
    w_in = np.ascontiguousarray(
        wh.reshape(KC, 128, P).transpose(1, 0, 2).reshape(128, KC * P))
    # Bases must come from the FULL-precision weights so each 128-step block
    # restarts at the reference-exact angle: the device's fp16-W error then
    # only drifts within one block instead of accumulating across the shard.
    return w_in, bh, bl, Wt, bt


def make_in_maps(x: np.ndarray, W: np.ndarray, b: np.ndarray):
    B, T, _ = x.shape
    w_in, bh, bl, w_eff, b_eff = prepare_weights(W, b)

    u_in = np.triu(np.ones((128, 128), np.float16))
    afs_in = np.stack([
        np.ones(128, np.float16), np.ones(128, np.float16),
        np.arange(1, 129, dtype=np.float16),
        np.arange(1, 129, dtype=np.float16)])

    # fp64 cumulative angle at every 128-step boundary, per batch (turns)
    nblk = T // BK                                              # 32
    xblk = x.reshape(B, nblk, BK, D).sum(axis=2, dtype=np.float64)
    dblk = xblk @ w_eff + BK * b_eff                            # [B, 32, P]
    bases = np.zeros((B, nblk, P))
    np.cumsum(dblk[:, :-1], axis=1, out=bases[:, 1:])           # exclusive

    in_maps = []
    for c in range(N_CORES):
        bb, hh = c // 2, c % 2
        xs = x[bb, hh * TL:(hh + 1) * TL, :].astype(np.float16)  # [TL, D]
        # xt: [g*128 + dp, (bkl*KC + dc)*128 + tl] = xs[(g*G+bkl)*128+tl,
        #                                              dc*128 + dp]
        xt_in = np.ascontiguousarray(
            xs.reshape(NG, G, BK, KC, 128).transpose(0, 4, 1, 3, 2)
            .reshape(NG * 128, G * KC * 128))
        # x12: [g*128 + tl, half*WID + bkl*P + p]
        x12_in = np.ascontiguousarray(
            xs[:, :ROT].reshape(NG, G, BK, 2, P).transpose(0, 2, 3, 1, 4)
            .reshape(NG * 128, 2 * WID))
        bs = bases[bb, hh * NBK:(hh + 1) * NBK]                 # [NBK, P]
        bs_hi = bs.astype(np.float16)
        bs_lo = (bs - bs_hi.astype(np.float64)).astype(np.float16)
        afm_in = np.stack([
            bs_hi.reshape(NBK * P), bs_lo.reshape(NBK * P),
            np.tile(bh, NBK), np.tile(bl, NBK)])
        in_maps.append({
            "xt": xt_in, "w": w_in, "u": u_in,
            "afs": afs_in, "afm": np.ascontiguousarray(afm_in),
            "x12": x12_in,
        })
    return in_maps


def assemble_output(x: np.ndarray, results) -> np.ndarray:
    B, T, Din = x.shape
    out = np.empty((B, T, Din), np.float32)
    out[:, :, ROT:] = x[:, :, ROT:]
    for c in range(N_CORES):
        bb, hh = c // 2, c % 2
        r = results[c]["outT"]                                  # [NG*128, 2*WID]
        blk = (r.reshape(NG, BK, 2, G, P).transpose(0, 3, 1, 2, 4)
               .reshape(TL, ROT))
        out[bb, hh * TL:(hh + 1) * TL, :ROT] = blk.astype(np.float32)
    return out


def kernel(x: np.ndarray, W: np.ndarray, b: np.ndarray) -> np.ndarray:
    nc = _get_nc()
    in_maps = make_in_maps(x, W, b)
    res = run_bass_kernel_spmd(nc, in_maps, list(range(N_CORES)))
    return assemble_output(x, res.results)

